# revision 10
# baseline (speedup 1.0000x reference)
"""MaxIoUAssigner on 8 Trainium2 NeuronCores (Bass/Tile).

Contract: kernel(bboxes[200000,4] f32, gt_bboxes[256,4] f32) -> assigned[200000] int32
Matches reference semantics:
  overlaps = iou(gt, priors)          [G, N]
  max/argmax per prior; negatives (<0.5) -> 0; positives (>=0.5) -> argmax+1
  low-quality: priors tying a gt's row max get gt_i+1 (later gt wins)

Sharding: priors split across 8 cores (25000 each, padded to 25088 = 196*128
with far-away dummy boxes). Per-gt row max needs an AllReduce(max) over the
prior shards; done on-device with a 1KB DRAM collective.

Layout: priors on partitions (128/tile), gts along the free dim (256).
GT coords live in partition-broadcast [128,256] tiles; prior coords are
per-partition scalars, so the IoU pipeline is fused tensor_scalar /
scalar_tensor_tensor ops. IoU tiles are stashed to DRAM between phases.
"""

import sys

if "/opt/trn_rl_repo" not in sys.path:
    sys.path.insert(0, "/opt/trn_rl_repo")

import numpy as np

from concourse import bacc, bass_utils, mybir, tile

f32 = mybir.dt.float32
i32 = mybir.dt.int32
u32 = mybir.dt.uint32
Alu = mybir.AluOpType

N_FULL = 200000
G = 256
N_CORES = 8
N_SHARD = N_FULL // N_CORES          # 25000
P = 128
TG = 4                               # sub-tiles ([128, G]) per group
NS = 25088                           # padded shard: 196 tiles of 128
PAD_BOX = (4000.0, 4000.0, 4001.0, 4001.0)  # zero-IoU dummy prior


def _build_program(ns=NS, n_cores=N_CORES, debug=False, repeat=1):
    import concourse.bass_isa as bass_isa

    TILES = ns // P
    GROUPS = TILES // TG
    nc = bacc.Bacc("TRN2", target_bir_lowering=False, debug=debug,
                   num_devices=n_cores)
    bb = nc.dram_tensor("bb", [ns, 4], f32, kind="ExternalInput").ap()
    gt = nc.dram_tensor("gt", [G, 4], f32, kind="ExternalInput").ap()
    out = nc.dram_tensor("assigned", [ns], i32, kind="ExternalOutput").ap()

    with tile.TileContext(nc) as tc:
        with (
            tc.tile_pool(name="const", bufs=1) as cpool,
            tc.tile_pool(name="work", bufs=2) as wpool,
            tc.tile_pool(name="io", bufs=3) as iopool,
            tc.tile_pool(name="dram", bufs=1, space="DRAM") as dpool,
        ):
            # ---- constants / staging ----
            gx1_t = cpool.tile([P, G], f32, tag="gx1")
            gy1_t = cpool.tile([P, G], f32, tag="gy1")
            gx2_t = cpool.tile([P, G], f32, tag="gx2")
            gy2_t = cpool.tile([P, G], f32, tag="gy2")
            ag_t = cpool.tile([P, G], f32, tag="ag")
            iotaf_t = cpool.tile([P, G], f32, tag="iotaf")
            gmax_acc = cpool.tile([P, G], f32, tag="gmax")
            gmax_b = cpool.tile([P, G], f32, tag="gmaxb")
            gtmax_t = cpool.tile([P, G], f32, tag="gtmax")
            grow = cpool.tile([1, G], f32, tag="grow")
            iotai_t = cpool.tile([P, G], i32, tag="iotai")
            scr_a = cpool.tile([P, G], f32, tag="scra")
            scr_b = cpool.tile([P, G], f32, tag="scrb")
            pmax_st = cpool.tile([P, TILES], f32, tag="pmaxst")
            parg_st = cpool.tile([P, TILES], f32, tag="pargst")
            lq_st = cpool.tile([P, TILES], f32, tag="lqst")
            comb_m = cpool.tile([P, TILES], f32, tag="combm")
            comb_v = cpool.tile([P, TILES], f32, tag="combv")
            out_i = cpool.tile([P, TILES], i32, tag="outi")

            stash = dpool.tile([ns, G], f32, tag="stash")
            cc_in = dpool.tile([1, G], f32, tag="ccin")
            cc_out = dpool.tile([1, G], f32, tag="ccout")

            # gt coord rows -> broadcast tiles
            for k, dst in ((0, gx1_t), (1, gy1_t), (2, gx2_t), (3, gy2_t)):
                nc.sync.dma_start(grow[:1, :],
                                  gt[:, k].rearrange("(o g) -> o g", o=1))
                nc.gpsimd.partition_broadcast(dst[:], grow[:1, :])
            # gt areas (same op order as reference: (x2-x1)*(y2-y1))
            nc.vector.tensor_sub(scr_a[:], gx2_t[:], gx1_t[:])
            nc.vector.tensor_sub(scr_b[:], gy2_t[:], gy1_t[:])
            nc.vector.tensor_mul(ag_t[:], scr_a[:], scr_b[:])
            # iota (1..G) as f32
            nc.gpsimd.iota(iotai_t[:], pattern=[[1, G]], base=1,
                           channel_multiplier=0)
            nc.vector.tensor_copy(iotaf_t[:], iotai_t[:])
            nc.gpsimd.memset(gmax_acc[:], 0.0)

            for _rep in range(repeat):
                # ---- phase 1: iou, per-prior max/argmax, per-gt max ----
                for g in range(GROUPS):
                    r0 = g * TG * P
                    bbt = iopool.tile([P, TG, 4], f32, tag="bbt")
                    nc.sync.dma_start(
                        bbt[:],
                        bb[r0:r0 + TG * P, :].rearrange("(t p) c -> p t c", p=P))
                    whab = iopool.tile([P, TG, 3], f32, tag="whab")
                    nc.gpsimd.tensor_sub(whab[:, :, 0], bbt[:, :, 2], bbt[:, :, 0])
                    nc.gpsimd.tensor_sub(whab[:, :, 1], bbt[:, :, 3], bbt[:, :, 1])
                    nc.gpsimd.tensor_mul(whab[:, :, 2], whab[:, :, 0], whab[:, :, 1])

                    ix_g = wpool.tile([P, TG, G], f32, tag="ix")
                    iy_g = wpool.tile([P, TG, G], f32, tag="iy")
                    s1_g = wpool.tile([P, TG, G], f32, tag="s1")
                    s2_g = wpool.tile([P, TG, G], f32, tag="s2")
                    t_g = wpool.tile([P, TG, G], f32, tag="t")
                    u_g = wpool.tile([P, TG, G], f32, tag="u")
                    r_g = wpool.tile([P, TG, G], f32, tag="r")
                    r2_g = wpool.tile([P, TG, G], f32, tag="r2")
                    iou_g = wpool.tile([P, TG, G], f32, tag="iou")
                    inmax8 = wpool.tile([P, TG, 8], f32, tag="inmax8")
                    idx8 = wpool.tile([P, TG, 8], u32, tag="idx8")

                    for t in range(TG):
                        bx1 = bbt[:, t, 0:1]
                        by1 = bbt[:, t, 1:2]
                        bx2 = bbt[:, t, 2:3]
                        by2 = bbt[:, t, 3:4]
                        ab = whab[:, t, 2:3]
                        # lt = max(gt[:2], b[:2]) ; rb = min(gt[2:], b[2:])
                        nc.vector.tensor_scalar(ix_g[:, t], gx1_t[:], bx1, None,
                                                op0=Alu.max)
                        nc.vector.tensor_scalar(iy_g[:, t], gy1_t[:], by1, None,
                                                op0=Alu.max)
                        # s = min(g2, b2) - lt   (matches rb - lt)
                        nc.vector.scalar_tensor_tensor(
                            s1_g[:, t], gx2_t[:], bx2, ix_g[:, t],
                            op0=Alu.min, op1=Alu.subtract)
                        nc.vector.scalar_tensor_tensor(
                            s2_g[:, t], gy2_t[:], by2, iy_g[:, t],
                            op0=Alu.min, op1=Alu.subtract)
                        # t = max(s1,0)*s2 (<=0 where no overlap)
                        nc.vector.scalar_tensor_tensor(
                            t_g[:, t], s1_g[:, t], 0.0, s2_g[:, t],
                            op0=Alu.max, op1=Alu.mult)
                        # u = (ag + ab) - t
                        nc.vector.scalar_tensor_tensor(
                            u_g[:, t], ag_t[:], ab, t_g[:, t],
                            op0=Alu.add, op1=Alu.subtract)

                    # r2 ~ 1/u at ~2 ulp (grouped over the whole [P, TG*G])
                    nc.vector.reciprocal_approx_accurate(
                        r2_g.rearrange("p t g -> p (t g)"),
                        u_g.rearrange("p t g -> p (t g)"),
                        r_g.rearrange("p t g -> p (t g)"))

                    for t in range(TG):
                        # iou = t*r2 (negative where no overlap; matches ref 0
                        # in every downstream comparison)
                        nc.vector.tensor_mul(iou_g[:, t], t_g[:, t], r2_g[:, t])
                        nc.vector.max(inmax8[:, t], iou_g[:, t])
                        nc.vector.max_index(idx8[:, t], inmax8[:, t],
                                            iou_g[:, t])
                    # per-gt running max: reduce TG sub-tiles, then fold in
                    gred = wpool.tile([P, G], f32, tag="gred")
                    nc.vector.tensor_reduce(
                        gred[:], iou_g[:].rearrange("p t g -> p g t"),
                        axis=mybir.AxisListType.X, op=Alu.max)
                    nc.vector.tensor_max(gmax_acc[:], gmax_acc[:], gred[:])

                    # stage per-prior results; stash iou to DRAM
                    nc.scalar.copy(pmax_st[:, g * TG:(g + 1) * TG],
                                   inmax8[:, :, 0])
                    nc.vector.tensor_copy(parg_st[:, g * TG:(g + 1) * TG],
                                          idx8[:, :, 0])
                    nc.sync.dma_start(
                        stash[r0:r0 + TG * P, :].rearrange(
                            "(t p) i -> p t i", p=P),
                        iou_g[:])

                # ---- all-reduce per-gt max across cores ----
                nc.gpsimd.partition_all_reduce(gmax_b[:], gmax_acc[:],
                                               channels=P,
                                               reduce_op=bass_isa.ReduceOp.max)
                nc.sync.dma_start(cc_in[:], gmax_b[0:1, :])
                nc.gpsimd.collective_compute(
                    "AllReduce", Alu.max,
                    replica_groups=[list(range(n_cores))],
                    ins=[cc_in[:].opt()], outs=[cc_out[:].opt()])
                nc.sync.dma_start(grow[:1, :], cc_out[:])
                nc.gpsimd.partition_broadcast(gtmax_t[:], grow[:1, :])

                # ---- phase 2: low-quality matches from stashed iou ----
                for g in range(GROUPS):
                    r0 = g * TG * P
                    iou2 = wpool.tile([P, TG, G], f32, tag="iou2")
                    msk = wpool.tile([P, TG, G], f32, tag="msk")
                    cand = wpool.tile([P, TG, G], f32, tag="cand")
                    nc.sync.dma_start(
                        iou2[:],
                        stash[r0:r0 + TG * P, :].rearrange(
                            "(t p) i -> p t i", p=P))
                    for t in range(TG):
                        # iou == gtmax  <=>  iou >= gtmax (iou <= gtmax always)
                        nc.vector.tensor_tensor(msk[:, t], iou2[:, t],
                                                gtmax_t[:], op=Alu.is_ge)
                        # lq = max_i mask*(i+1) -> later gt wins, 0 if none
                        nc.vector.tensor_mul(cand[:, t], msk[:, t], iotaf_t[:])
                        nc.vector.tensor_reduce(
                            lq_st[:, g * TG + t:g * TG + t + 1], cand[:, t],
                            axis=mybir.AxisListType.X, op=Alu.max)

            # ---- combine: lq > 0 ? lq : (pmax >= 0.5 ? argmax+1 : 0) ----
            nc.vector.tensor_scalar(comb_m[:], pmax_st[:], 0.5, None,
                                    op0=Alu.is_ge)
            nc.vector.scalar_tensor_tensor(comb_v[:], parg_st[:], 1.0,
                                           comb_m[:], op0=Alu.add, op1=Alu.mult)
            # nolq = (lq < 1); out = pos*nolq + lq   (lq is 0 or >=1)
            nc.vector.tensor_scalar(comb_m[:], lq_st[:], 1.0, None,
                                    op0=Alu.is_lt)
            nc.vector.tensor_mul(comb_v[:], comb_v[:], comb_m[:])
            nc.vector.tensor_add(comb_v[:], comb_v[:], lq_st[:])
            nc.vector.tensor_copy(out_i[:], comb_v[:])
            nc.sync.dma_start(out.rearrange("(t p) -> p t", p=P), out_i[:])

    nc.compile()
    return nc


_NC_CACHE = None


def _get_program():
    global _NC_CACHE
    if _NC_CACHE is None:
        _NC_CACHE = _build_program()
    return _NC_CACHE


def kernel(bboxes: np.ndarray, gt_bboxes: np.ndarray) -> np.ndarray:
    assert bboxes.shape == (N_FULL, 4) and gt_bboxes.shape == (G, 4)
    nc = _get_program()

    bboxes = np.ascontiguousarray(bboxes, dtype=np.float32)
    gt = np.ascontiguousarray(gt_bboxes, dtype=np.float32)
    pad = np.tile(np.array(PAD_BOX, dtype=np.float32), (NS - N_SHARD, 1))
    in_maps = []
    for c in range(N_CORES):
        shard = bboxes[c * N_SHARD:(c + 1) * N_SHARD]
        in_maps.append({"bb": np.concatenate([shard, pad], axis=0), "gt": gt})

    res = bass_utils.run_bass_kernel_spmd(nc, in_maps,
                                          core_ids=list(range(N_CORES)))
    outs = [res.results[c]["assigned"][:N_SHARD] for c in range(N_CORES)]
    return np.concatenate(outs).astype(np.int32)


if __name__ == "__main__":
    rng = np.random.default_rng(0)
    bb = np.zeros((N_FULL, 4), np.float32)
    bb[:, :2] = rng.uniform(0, 928, (N_FULL, 2))
    bb[:, 2:] = bb[:, :2] + rng.uniform(1, 97, (N_FULL, 2))
    gtb = np.zeros((G, 4), np.float32)
    gtb[:, :2] = rng.uniform(0, 928, (G, 2))
    gtb[:, 2:] = gtb[:, :2] + rng.uniform(1, 97, (G, 2))
    out = kernel(bb, gtb)
    print("out", out.shape, out.dtype, out[:20])


# revision 11
# speedup vs baseline: 1.1966x; 1.1966x over previous
"""MaxIoUAssigner on 8 Trainium2 NeuronCores (Bass/Tile).

kernel(bboxes[200000,4] f32, gt_bboxes[256,4] f32) -> assigned[200000] int32

Reference semantics reproduced exactly:
  overlaps = iou(gt, priors)  [G=256, N=200000]
  per-prior max/argmax (first index wins ties); < 0.5 -> 0; >= 0.5 -> argmax+1
  low-quality: priors tying a gt's row max get gt_i+1 (later gt wins)

Distribution: priors sharded across 8 cores (25000 each, padded to 25600 =
10 chunks of 2560 with far-away zero-IoU dummy boxes). The per-gt row max
needs a cross-shard reduction: done on-device with a 1 KB DRAM AllReduce(max).

Layout (chosen for this platform's per-instruction-dominated cost model):
  - 256 gts -> 2 partition blocks of 128; gt coords/areas are per-partition
    scalars, so the whole IoU pipeline is fused tensor_scalar /
    scalar_tensor_tensor ops over [128, 2560] tiles.
  - prior coords+areas (areas precomputed on host, bit-identical f32) are
    0-stride-broadcast DMA'd into [128, 5, 2560] tiles: one DMA per chunk.
  - per-gt max = free-dim reduce; per-prior max / argmax / low-quality
    labels = partition_all_reduce (one GPSIMD instr per chunk each).
  - argmax-first tie-break: max over (256-g)*[iou==pmax]; low-quality
    later-gt-wins: max over (g+1)*[iou==gtmax].
  - IoU tiles stashed to DRAM between the two phases; exact (bit-accurate)
    nc.vector.reciprocal for the division.
"""

import sys

if "/opt/trn_rl_repo" not in sys.path:
    sys.path.insert(0, "/opt/trn_rl_repo")

import numpy as np

from concourse import bacc, bass_utils, mybir, tile

f32 = mybir.dt.float32
i32 = mybir.dt.int32
Alu = mybir.AluOpType

N_FULL = 200000
G = 256
GB = 2                               # gt partition blocks
P = 128
N_CORES = 8
N_SHARD = N_FULL // N_CORES          # 25000
F = 2560                             # priors per chunk
NS = 25600                           # padded shard (10 chunks)
PAD_BOX = (4000.0, 4000.0, 4001.0, 4001.0)


def build_program(ns=NS, n_cores=N_CORES, repeat=1, f=F):
    import concourse.bass_isa as bass_isa

    chunks = ns // f
    fs = f // P
    TS_ = chunks * fs
    nc = bacc.Bacc("TRN2", target_bir_lowering=False, debug=False,
                   num_devices=n_cores)
    bb = nc.dram_tensor("bb", [5, ns], f32, kind="ExternalInput").ap()
    gt = nc.dram_tensor("gt", [G, 4], f32, kind="ExternalInput").ap()
    out = nc.dram_tensor("assigned", [ns], i32, kind="ExternalOutput").ap()

    with tile.TileContext(nc) as tc:
        with (
            tc.tile_pool(name="const", bufs=1) as cpool,
            tc.tile_pool(name="work", bufs=1) as wpool,
            tc.tile_pool(name="rows", bufs=2) as rpool,
            tc.tile_pool(name="dram", bufs=1, space="DRAM") as dpool,
        ):
            # ---- constants ----
            gtc = cpool.tile([P, GB, 4], f32, tag="gtc")
            agc = cpool.tile([P, GB], f32, tag="agc")
            gw = cpool.tile([P, GB], f32, tag="gw")
            gh = cpool.tile([P, GB], f32, tag="gh")
            wrev_i = cpool.tile([P, GB], i32, tag="wrevi")
            wrev = cpool.tile([P, GB], f32, tag="wrev")
            gp1_i = cpool.tile([P, GB], i32, tag="gp1i")
            gp1 = cpool.tile([P, GB], f32, tag="gp1")
            gacc = cpool.tile([P, GB], f32, tag="gacc")
            gtmaxc = cpool.tile([P, GB], f32, tag="gtmaxc")
            pm_st = cpool.tile([P, TS_], f32, tag="pmst")
            am_st = cpool.tile([P, TS_], f32, tag="amst")
            lq_st = cpool.tile([P, TS_], f32, tag="lqst")
            cmb_m = cpool.tile([P, TS_], f32, tag="cmbm")
            cmb_v = cpool.tile([P, TS_], f32, tag="cmbv")
            out_i = cpool.tile([P, TS_], i32, tag="outi")

            stash = dpool.tile([G, ns], f32, tag="stash")
            st_dram = dpool.tile([3, ns], f32, tag="stdram")
            cc_in = dpool.tile([1, G], f32, tag="ccin")
            cc_out = dpool.tile([1, G], f32, tag="ccout")

            def bc1(col2, n):
                # [P, GB, n] 0-step-broadcast view of a [P, GB] column pair
                return (col2.rearrange("p (b o) -> p b o", o=1)
                        .broadcast_to([P, GB, n]))

            # gt g = b*128+p -> per-partition scalars
            nc.sync.dma_start(gtc[:], gt.rearrange("(b p) c -> p b c", p=P))
            nc.vector.tensor_sub(gw[:], gtc[:, :, 2], gtc[:, :, 0])
            nc.vector.tensor_sub(gh[:], gtc[:, :, 3], gtc[:, :, 1])
            nc.vector.tensor_mul(agc[:], gw[:], gh[:])
            # wrev[p,b] = 256-(b*128+p); gp1[p,b] = b*128+p+1
            nc.gpsimd.iota(wrev_i[:], pattern=[[-P, GB]], base=G,
                           channel_multiplier=-1)
            nc.vector.tensor_copy(wrev[:], wrev_i[:])
            nc.gpsimd.iota(gp1_i[:], pattern=[[P, GB]], base=1,
                           channel_multiplier=1)
            nc.vector.tensor_copy(gp1[:], gp1_i[:])
            nc.gpsimd.memset(gacc[:], 0.0)

            for _rep in range(repeat):
                # ---- phase 1: iou, per-gt max, per-prior max/argmax ----
                for c in range(chunks):
                    col = slice(c * f, (c + 1) * f)
                    b5 = wpool.tile([P, 5, f], f32, tag="b5")
                    nc.sync.dma_start(
                        b5[:], bb[:, col].rearrange("(o c) n -> o c n", o=1)
                        .broadcast_to([P, 5, f]))
                    bx1_t, by1_t = b5[:, 0], b5[:, 1]
                    bx2_t, by2_t = b5[:, 2], b5[:, 3]
                    ab_t = b5[:, 4]

                    ix_t = wpool.tile([P, f], f32, tag="ix")
                    iy_t = wpool.tile([P, f], f32, tag="iy")
                    s1_t = wpool.tile([P, f], f32, tag="s1")
                    s2_t = wpool.tile([P, f], f32, tag="s2")
                    t_a = wpool.tile([P, GB, f], f32, tag="ta")
                    u_a = wpool.tile([P, GB, f], f32, tag="ua")
                    r_a = wpool.tile([P, GB, f], f32, tag="ra")
                    iou_a = wpool.tile([P, GB, f], f32, tag="ioua")

                    for b in range(GB):
                        gx1 = gtc[:, b, 0:1]
                        gy1 = gtc[:, b, 1:2]
                        gx2 = gtc[:, b, 2:3]
                        gy2 = gtc[:, b, 3:4]
                        # lt = max(gt[:2], prior[:2])
                        nc.vector.tensor_scalar(ix_t[:], bx1_t, gx1, None,
                                                op0=Alu.max)
                        nc.vector.tensor_scalar(iy_t[:], by1_t, gy1, None,
                                                op0=Alu.max)
                        # s = min(gt[2:], prior[2:]) - lt
                        nc.vector.scalar_tensor_tensor(
                            s1_t[:], bx2_t, gx2, ix_t[:],
                            op0=Alu.min, op1=Alu.subtract)
                        nc.vector.scalar_tensor_tensor(
                            s2_t[:], by2_t, gy2, iy_t[:],
                            op0=Alu.min, op1=Alu.subtract)
                        # t = max(s1,0)*s2 (<=0 where no overlap; every
                        # downstream comparison matches reference's 0)
                        nc.vector.scalar_tensor_tensor(
                            t_a[:, b], s1_t[:], 0.0, s2_t[:],
                            op0=Alu.max, op1=Alu.mult)
                        # u = (area_b + area_g) - t  (f32 add commutes bitwise)
                        nc.vector.scalar_tensor_tensor(
                            u_a[:, b], ab_t, agc[:, b:b + 1], t_a[:, b],
                            op0=Alu.add, op1=Alu.subtract)

                    nc.vector.reciprocal(r_a.rearrange("p b n -> p (b n)"),
                                         u_a.rearrange("p b n -> p (b n)"))
                    nc.vector.tensor_mul(iou_a[:], t_a[:], r_a[:])

                    # per-gt running max
                    gred = rpool.tile([P, GB], f32, tag="gred")
                    nc.vector.tensor_reduce(gred[:], iou_a[:],
                                            axis=mybir.AxisListType.X,
                                            op=Alu.max)
                    nc.vector.tensor_max(gacc[:], gacc[:], gred[:])

                    # stash iou (gt-major [256, ns]) for phase 2
                    nc.sync.dma_start(
                        stash[:, col].rearrange("(b p) n -> p b n", p=P),
                        iou_a[:])

                    # per-prior max over gts
                    pr_a = wpool.tile([P, GB, f], f32, tag="pra")
                    nc.gpsimd.partition_all_reduce(
                        pr_a.rearrange("p b n -> p (b n)"),
                        iou_a.rearrange("p b n -> p (b n)"),
                        channels=P, reduce_op=bass_isa.ReduceOp.max)
                    pmax_t = wpool.tile([P, f], f32, tag="pmax")
                    nc.vector.tensor_max(pmax_t[:], pr_a[:, 0], pr_a[:, 1])

                    # argmax-first: max of (256-g)*[iou==pmax]
                    msk_a = wpool.tile([P, GB, f], f32, tag="pra")
                    nc.vector.tensor_tensor(
                        msk_a[:], iou_a[:],
                        pmax_t[:].rearrange("p (o n) -> p o n", o=1)
                        .broadcast_to([P, GB, f]),
                        op=Alu.is_ge)
                    nc.vector.tensor_mul(msk_a[:], msk_a[:], bc1(wrev[:], f))
                    nc.gpsimd.partition_all_reduce(
                        msk_a.rearrange("p b n -> p (b n)"),
                        msk_a.rearrange("p b n -> p (b n)"),
                        channels=P, reduce_op=bass_isa.ReduceOp.max)
                    am_t = wpool.tile([P, f], f32, tag="ix")
                    nc.vector.tensor_max(am_t[:], msk_a[:, 0], msk_a[:, 1])

                    # stage result rows to DRAM (row 0 holds the answer)
                    nc.sync.dma_start(st_dram[0:1, col], pmax_t[0:1, :])
                    nc.sync.dma_start(st_dram[1:2, col], am_t[0:1, :])

                # ---- all-reduce per-gt max across the 8 cores ----
                nc.sync.dma_start(
                    cc_in.rearrange("o (b p) -> (o p) b", p=P), gacc[:])
                nc.gpsimd.collective_compute(
                    "AllReduce", Alu.max,
                    replica_groups=[list(range(n_cores))],
                    ins=[cc_in[:].opt()], outs=[cc_out[:].opt()])
                nc.sync.dma_start(
                    gtmaxc[:], cc_out.rearrange("o (b p) -> (o p) b", p=P))

                # ---- phase 2: low-quality matches from stashed iou ----
                for c in range(chunks):
                    col = slice(c * f, (c + 1) * f)
                    iou_a = wpool.tile([P, GB, f], f32, tag="ioua")
                    cd_a = wpool.tile([P, GB, f], f32, tag="pra")
                    nc.sync.dma_start(
                        iou_a[:],
                        stash[:, col].rearrange("(b p) n -> p b n", p=P))
                    # iou == gtmax  <=>  iou >= gtmax (iou <= gtmax always)
                    nc.vector.tensor_tensor(cd_a[:], iou_a[:],
                                            bc1(gtmaxc[:], f), op=Alu.is_ge)
                    nc.vector.tensor_mul(cd_a[:], cd_a[:], bc1(gp1[:], f))
                    nc.gpsimd.partition_all_reduce(
                        cd_a.rearrange("p b n -> p (b n)"),
                        cd_a.rearrange("p b n -> p (b n)"),
                        channels=P, reduce_op=bass_isa.ReduceOp.max)
                    lq_t = wpool.tile([P, f], f32, tag="pmax")
                    nc.vector.tensor_max(lq_t[:], cd_a[:, 0], cd_a[:, 1])
                    nc.sync.dma_start(st_dram[2:3, col], lq_t[0:1, :])

            # reload staged rows as [128, chunks*fs]
            for v, tl in ((0, pm_st), (1, am_st), (2, lq_st)):
                nc.sync.dma_start(
                    tl[:].rearrange("p (c f) -> p c f", f=fs),
                    st_dram[v, :].rearrange("(c p f) -> p c f", p=P, f=fs))

            # ---- combine: lq > 0 ? lq : (pmax >= 0.5 ? (257-am) : 0) ----
            nc.vector.tensor_scalar(cmb_m[:], pm_st[:], 0.5, None,
                                    op0=Alu.is_ge)
            nc.vector.tensor_scalar(cmb_v[:], am_st[:], -1.0, float(G + 1),
                                    op0=Alu.mult, op1=Alu.add)
            nc.vector.tensor_mul(cmb_v[:], cmb_v[:], cmb_m[:])
            nc.vector.tensor_scalar(cmb_m[:], lq_st[:], 1.0, None,
                                    op0=Alu.is_lt)
            nc.vector.tensor_mul(cmb_v[:], cmb_v[:], cmb_m[:])
            nc.vector.tensor_add(cmb_v[:], cmb_v[:], lq_st[:])
            nc.vector.tensor_copy(out_i[:], cmb_v[:])
            nc.sync.dma_start(
                out.rearrange("(c p f) -> p c f", p=P, f=fs),
                out_i[:].rearrange("p (c f) -> p c f", f=fs))

    nc.compile()
    return nc


def make_bbx(shard_boxes, ns):
    """[n,4] f32 -> [5, ns]: rows x1,y1,x2,y2,area; PAD_BOX padding."""
    n = shard_boxes.shape[0]
    bbx = np.empty((5, ns), np.float32)
    bbx[0, :n] = shard_boxes[:, 0]
    bbx[1, :n] = shard_boxes[:, 1]
    bbx[2, :n] = shard_boxes[:, 2]
    bbx[3, :n] = shard_boxes[:, 3]
    pb = np.array(PAD_BOX, np.float32)
    bbx[0, n:], bbx[1, n:], bbx[2, n:], bbx[3, n:] = pb[0], pb[1], pb[2], pb[3]
    bbx[4] = (bbx[2] - bbx[0]) * (bbx[3] - bbx[1])
    return bbx


_NC_CACHE = None


def _get_program():
    global _NC_CACHE
    if _NC_CACHE is None:
        _NC_CACHE = build_program()
    return _NC_CACHE


def kernel(bboxes: np.ndarray, gt_bboxes: np.ndarray) -> np.ndarray:
    assert bboxes.shape == (N_FULL, 4) and gt_bboxes.shape == (G, 4)
    nc = _get_program()

    bboxes = np.ascontiguousarray(bboxes, dtype=np.float32)
    gt = np.ascontiguousarray(gt_bboxes, dtype=np.float32)
    in_maps = []
    for c in range(N_CORES):
        shard = bboxes[c * N_SHARD:(c + 1) * N_SHARD]
        in_maps.append({"bb": make_bbx(shard, NS), "gt": gt})

    res = bass_utils.run_bass_kernel_spmd(nc, in_maps,
                                          core_ids=list(range(N_CORES)))
    outs = [res.results[c]["assigned"][:N_SHARD] for c in range(N_CORES)]
    return np.concatenate(outs).astype(np.int32)


if __name__ == "__main__":
    rng = np.random.default_rng(0)
    bb_ = np.zeros((N_FULL, 4), np.float32)
    bb_[:, :2] = rng.uniform(0, 928, (N_FULL, 2))
    bb_[:, 2:] = bb_[:, :2] + rng.uniform(1, 97, (N_FULL, 2))
    gtb = np.zeros((G, 4), np.float32)
    gtb[:, :2] = rng.uniform(0, 928, (G, 2))
    gtb[:, 2:] = gtb[:, :2] + rng.uniform(1, 97, (G, 2))
    print(kernel(bb_, gtb)[:20])


# revision 15
# speedup vs baseline: 3.0744x; 2.5694x over previous
"""MaxIoUAssigner on 8 Trainium2 NeuronCores (Bass/Tile).

kernel(bboxes[200000,4] f32, gt_bboxes[256,4] f32) -> assigned[200000] int32

Reference semantics reproduced exactly:
  overlaps = iou(gt, priors)  [G=256, N=200000]
  per-prior max/argmax (first index wins ties); < 0.5 -> 0; >= 0.5 -> argmax+1
  low-quality: priors tying a gt's row max get gt_i+1 (later gt wins)

Distribution: priors sharded across 8 cores (25000 each, padded to 25600 =
10 chunks of 2560 with far-away zero-IoU dummy boxes). The per-gt row max
needs a cross-shard reduction: done on-device with a 1 KB DRAM AllReduce(max).

Layout (chosen for this platform's per-instruction-dominated cost model):
  - 256 gts -> 2 partition blocks of 128; gt coords/areas are per-partition
    scalars, so the whole IoU pipeline is fused tensor_scalar /
    scalar_tensor_tensor ops over [128, 2560] tiles.
  - prior coords+areas (areas precomputed on host, bit-identical f32) are
    0-stride-broadcast DMA'd into [128, 5, 2560] tiles: one DMA per chunk.
  - per-gt max = free-dim reduce; per-prior max / argmax / low-quality
    labels = partition_all_reduce (one GPSIMD instr per chunk each).
  - argmax-first tie-break: max over (256-g)*[iou==pmax]; low-quality
    later-gt-wins: max over (g+1)*[iou==gtmax].
  - IoU tiles stashed to DRAM between the two phases; exact (bit-accurate)
    nc.vector.reciprocal for the division.
"""

import sys

if "/opt/trn_rl_repo" not in sys.path:
    sys.path.insert(0, "/opt/trn_rl_repo")

import numpy as np

from concourse import bacc, bass_utils, mybir, tile

f32 = mybir.dt.float32
i32 = mybir.dt.int32
Alu = mybir.AluOpType

N_FULL = 200000
G = 256
GB = 2                               # gt partition blocks
P = 128
N_CORES = 8
N_SHARD = N_FULL // N_CORES          # 25000
F = 3200                             # priors per chunk
NS = 25600                           # padded shard (8 chunks)
PAD_BOX = (4000.0, 4000.0, 4001.0, 4001.0)


def build_program(ns=NS, n_cores=N_CORES, repeat=1, f=F):
    import concourse.bass_isa as bass_isa

    chunks = ns // f
    fs = f // P
    TS_ = chunks * fs
    nc = bacc.Bacc("TRN2", target_bir_lowering=False, debug=False,
                   num_devices=n_cores)
    bb = nc.dram_tensor("bb", [5, ns], f32, kind="ExternalInput").ap()
    gt = nc.dram_tensor("gt", [G, 4], f32, kind="ExternalInput").ap()
    out = nc.dram_tensor("assigned", [ns], i32, kind="ExternalOutput").ap()

    with tile.TileContext(nc) as tc:
        with (
            tc.tile_pool(name="const", bufs=1) as cpool,
            tc.tile_pool(name="work", bufs=1) as wpool,
            tc.tile_pool(name="rows", bufs=2) as rpool,
            tc.tile_pool(name="dram", bufs=1, space="DRAM") as dpool,
        ):
            # ---- constants ----
            gtc = cpool.tile([P, GB, 4], f32, tag="gtc")
            agc = cpool.tile([P, GB], f32, tag="agc")
            gw = cpool.tile([P, GB], f32, tag="gw")
            gh = cpool.tile([P, GB], f32, tag="gh")
            wrev_i = cpool.tile([P, GB], i32, tag="wrevi")
            wrev = cpool.tile([P, GB], f32, tag="wrev")
            gp1_i = cpool.tile([P, GB], i32, tag="gp1i")
            gp1 = cpool.tile([P, GB], f32, tag="gp1")
            gacc = cpool.tile([P, GB], f32, tag="gacc")
            gtmaxc = cpool.tile([P, GB], f32, tag="gtmaxc")
            pm_st = cpool.tile([P, TS_], f32, tag="pmst")
            am_st = cpool.tile([P, TS_], f32, tag="amst")
            lq_st = cpool.tile([P, TS_], f32, tag="lqst")
            cmb_m = cpool.tile([P, TS_], f32, tag="cmbm")
            cmb_v = cpool.tile([P, TS_], f32, tag="cmbv")
            out_i = cpool.tile([P, TS_], i32, tag="outi")

            stash = dpool.tile([G, ns], f32, tag="stash")
            st_dram = dpool.tile([3, ns], f32, tag="stdram")
            cc_in = dpool.tile([1, G], f32, tag="ccin")
            cc_out = dpool.tile([1, G], f32, tag="ccout")

            def bc1(col2, n):
                # [P, GB, n] 0-step-broadcast view of a [P, GB] column pair
                return (col2.rearrange("p (b o) -> p b o", o=1)
                        .broadcast_to([P, GB, n]))

            # gt g = b*128+p -> per-partition scalars
            nc.sync.dma_start(gtc[:], gt.rearrange("(b p) c -> p b c", p=P))
            nc.vector.tensor_sub(gw[:], gtc[:, :, 2], gtc[:, :, 0])
            nc.vector.tensor_sub(gh[:], gtc[:, :, 3], gtc[:, :, 1])
            nc.vector.tensor_mul(agc[:], gw[:], gh[:])
            # wrev[p,b] = 256-(b*128+p); gp1[p,b] = b*128+p+1
            nc.gpsimd.iota(wrev_i[:], pattern=[[-P, GB]], base=G,
                           channel_multiplier=-1)
            nc.vector.tensor_copy(wrev[:], wrev_i[:])
            nc.gpsimd.iota(gp1_i[:], pattern=[[P, GB]], base=1,
                           channel_multiplier=1)
            nc.vector.tensor_copy(gp1[:], gp1_i[:])
            nc.gpsimd.memset(gacc[:], 0.0)

            for _rep in range(repeat):
                # ---- phase 1: iou, per-gt max, per-prior max/argmax ----
                for c in range(chunks):
                    col = slice(c * f, (c + 1) * f)
                    b5 = wpool.tile([P, 5, f], f32, tag="b5")
                    nc.sync.dma_start(
                        b5[:], bb[:, col].rearrange("(o c) n -> o c n", o=1)
                        .broadcast_to([P, 5, f]))
                    bx1_t, by1_t = b5[:, 0], b5[:, 1]
                    bx2_t, by2_t = b5[:, 2], b5[:, 3]
                    ab_t = b5[:, 4]

                    ix_t = wpool.tile([P, f], f32, tag="ix")
                    iy_t = wpool.tile([P, f], f32, tag="iy")
                    s1_t = wpool.tile([P, f], f32, tag="s1")
                    s2_t = wpool.tile([P, f], f32, tag="s2")
                    t_a = wpool.tile([P, GB, f], f32, tag="ta")
                    u_a = wpool.tile([P, GB, f], f32, tag="ua")
                    r_a = wpool.tile([P, GB, f], f32, tag="b5")
                    iou_a = wpool.tile([P, GB, f], f32, tag="ioua")

                    for b in range(GB):
                        gx1 = gtc[:, b, 0:1]
                        gy1 = gtc[:, b, 1:2]
                        gx2 = gtc[:, b, 2:3]
                        gy2 = gtc[:, b, 3:4]
                        # lt = max(gt[:2], prior[:2])
                        nc.vector.tensor_scalar(ix_t[:], bx1_t, gx1, None,
                                                op0=Alu.max)
                        nc.vector.tensor_scalar(iy_t[:], by1_t, gy1, None,
                                                op0=Alu.max)
                        # s = min(gt[2:], prior[2:]) - lt
                        nc.vector.scalar_tensor_tensor(
                            s1_t[:], bx2_t, gx2, ix_t[:],
                            op0=Alu.min, op1=Alu.subtract)
                        nc.vector.scalar_tensor_tensor(
                            s2_t[:], by2_t, gy2, iy_t[:],
                            op0=Alu.min, op1=Alu.subtract)
                        # t = max(s1,0)*s2 (<=0 where no overlap; every
                        # downstream comparison matches reference's 0)
                        nc.vector.scalar_tensor_tensor(
                            t_a[:, b], s1_t[:], 0.0, s2_t[:],
                            op0=Alu.max, op1=Alu.mult)
                        # u = (area_b + area_g) - t  (f32 add commutes bitwise)
                        nc.vector.scalar_tensor_tensor(
                            u_a[:, b], ab_t, agc[:, b:b + 1], t_a[:, b],
                            op0=Alu.add, op1=Alu.subtract)

                    nc.vector.reciprocal(r_a.rearrange("p b n -> p (b n)"),
                                         u_a.rearrange("p b n -> p (b n)"))
                    nc.vector.tensor_mul(iou_a[:], t_a[:], r_a[:])

                    # per-gt running max
                    gred = rpool.tile([P, GB], f32, tag="gred")
                    nc.vector.tensor_reduce(gred[:], iou_a[:],
                                            axis=mybir.AxisListType.X,
                                            op=Alu.max)
                    nc.vector.tensor_max(gacc[:], gacc[:], gred[:])

                    # stash iou (gt-major [256, ns]) for phase 2
                    nc.sync.dma_start(
                        stash[:, col].rearrange("(b p) n -> p b n", p=P),
                        iou_a[:])

                    # per-prior max over gts
                    pr_a = wpool.tile([P, GB, f], f32, tag="b5")
                    nc.gpsimd.partition_all_reduce(
                        pr_a.rearrange("p b n -> p (b n)"),
                        iou_a.rearrange("p b n -> p (b n)"),
                        channels=P, reduce_op=bass_isa.ReduceOp.max)
                    pam = wpool.tile([P, 2, f], f32, tag="ua")
                    pmax_t = pam[:, 0]
                    nc.vector.tensor_max(pmax_t, pr_a[:, 0], pr_a[:, 1])

                    # argmax-first: max of (256-g)*[iou==pmax]
                    msk_a = wpool.tile([P, GB, f], f32, tag="b5")
                    nc.vector.tensor_tensor(
                        msk_a[:], iou_a[:],
                        pmax_t.rearrange("p (o n) -> p o n", o=1)
                        .broadcast_to([P, GB, f]),
                        op=Alu.is_ge)
                    nc.vector.tensor_mul(msk_a[:], msk_a[:], bc1(wrev[:], f))
                    nc.gpsimd.partition_all_reduce(
                        msk_a.rearrange("p b n -> p (b n)"),
                        msk_a.rearrange("p b n -> p (b n)"),
                        channels=P, reduce_op=bass_isa.ReduceOp.max)
                    nc.vector.tensor_max(pam[:, 1], msk_a[:, 0], msk_a[:, 1])

                    # stage pmax+argmax rows in one DMA (row 0 = full result)
                    nc.sync.dma_start(
                        st_dram[0:2, col].rearrange("(o b) n -> o b n", o=1),
                        pam[0:1, :, :])

                # ---- all-reduce per-gt max across the 8 cores ----
                nc.sync.dma_start(
                    cc_in.rearrange("o (b p) -> (o p) b", p=P), gacc[:])
                nc.gpsimd.collective_compute(
                    "AllReduce", Alu.max,
                    replica_groups=[list(range(n_cores))],
                    ins=[cc_in[:].opt()], outs=[cc_out[:].opt()])
                nc.sync.dma_start(
                    gtmaxc[:], cc_out.rearrange("o (b p) -> (o p) b", p=P))

                # ---- phase 2: low-quality matches from stashed iou ----
                for c in range(chunks):
                    col = slice(c * f, (c + 1) * f)
                    iou_a = wpool.tile([P, GB, f], f32, tag="ioua")
                    cd_a = wpool.tile([P, GB, f], f32, tag="b5")
                    nc.sync.dma_start(
                        iou_a[:],
                        stash[:, col].rearrange("(b p) n -> p b n", p=P))
                    # iou == gtmax  <=>  iou >= gtmax (iou <= gtmax always)
                    nc.vector.tensor_tensor(cd_a[:], iou_a[:],
                                            bc1(gtmaxc[:], f), op=Alu.is_ge)
                    nc.vector.tensor_mul(cd_a[:], cd_a[:], bc1(gp1[:], f))
                    nc.gpsimd.partition_all_reduce(
                        cd_a.rearrange("p b n -> p (b n)"),
                        cd_a.rearrange("p b n -> p (b n)"),
                        channels=P, reduce_op=bass_isa.ReduceOp.max)
                    lq_t = wpool.tile([P, 2, f], f32, tag="ua")
                    nc.vector.tensor_max(lq_t[:, 0], cd_a[:, 0], cd_a[:, 1])
                    nc.sync.dma_start(st_dram[2:3, col], lq_t[0:1, 0, :])

            # reload staged rows as [128, chunks*fs]
            for v, tl in ((0, pm_st), (1, am_st), (2, lq_st)):
                nc.sync.dma_start(
                    tl[:].rearrange("p (c f) -> p c f", f=fs),
                    st_dram[v, :].rearrange("(c p f) -> p c f", p=P, f=fs))

            # ---- combine: lq > 0 ? lq : (pmax >= 0.5 ? (257-am) : 0) ----
            nc.vector.tensor_scalar(cmb_m[:], pm_st[:], 0.5, None,
                                    op0=Alu.is_ge)
            nc.vector.tensor_scalar(cmb_v[:], am_st[:], -1.0, float(G + 1),
                                    op0=Alu.mult, op1=Alu.add)
            nc.vector.tensor_mul(cmb_v[:], cmb_v[:], cmb_m[:])
            nc.vector.tensor_scalar(cmb_m[:], lq_st[:], 1.0, None,
                                    op0=Alu.is_lt)
            nc.vector.tensor_mul(cmb_v[:], cmb_v[:], cmb_m[:])
            nc.vector.tensor_add(cmb_v[:], cmb_v[:], lq_st[:])
            nc.vector.tensor_copy(out_i[:], cmb_v[:])
            nc.sync.dma_start(
                out.rearrange("(c p f) -> p c f", p=P, f=fs),
                out_i[:].rearrange("p (c f) -> p c f", f=fs))

    nc.compile()
    return nc


def make_bbx(shard_boxes, ns):
    """[n,4] f32 -> [5, ns]: rows x1,y1,x2,y2,area; PAD_BOX padding."""
    n = shard_boxes.shape[0]
    bbx = np.empty((5, ns), np.float32)
    bbx[0, :n] = shard_boxes[:, 0]
    bbx[1, :n] = shard_boxes[:, 1]
    bbx[2, :n] = shard_boxes[:, 2]
    bbx[3, :n] = shard_boxes[:, 3]
    pb = np.array(PAD_BOX, np.float32)
    bbx[0, n:], bbx[1, n:], bbx[2, n:], bbx[3, n:] = pb[0], pb[1], pb[2], pb[3]
    bbx[4] = (bbx[2] - bbx[0]) * (bbx[3] - bbx[1])
    return bbx


_NC_CACHE = None


def _get_program():
    global _NC_CACHE
    if _NC_CACHE is None:
        _NC_CACHE = build_program()
    return _NC_CACHE


def kernel(bboxes: np.ndarray, gt_bboxes: np.ndarray) -> np.ndarray:
    assert bboxes.shape == (N_FULL, 4) and gt_bboxes.shape == (G, 4)
    nc = _get_program()

    bboxes = np.ascontiguousarray(bboxes, dtype=np.float32)
    gt = np.ascontiguousarray(gt_bboxes, dtype=np.float32)
    in_maps = []
    for c in range(N_CORES):
        shard = bboxes[c * N_SHARD:(c + 1) * N_SHARD]
        in_maps.append({"bb": make_bbx(shard, NS), "gt": gt})

    res = bass_utils.run_bass_kernel_spmd(nc, in_maps,
                                          core_ids=list(range(N_CORES)))
    outs = [res.results[c]["assigned"][:N_SHARD] for c in range(N_CORES)]
    return np.concatenate(outs).astype(np.int32)


if __name__ == "__main__":
    rng = np.random.default_rng(0)
    bb_ = np.zeros((N_FULL, 4), np.float32)
    bb_[:, :2] = rng.uniform(0, 928, (N_FULL, 2))
    bb_[:, 2:] = bb_[:, :2] + rng.uniform(1, 97, (N_FULL, 2))
    gtb = np.zeros((G, 4), np.float32)
    gtb[:, :2] = rng.uniform(0, 928, (G, 2))
    gtb[:, 2:] = gtb[:, :2] + rng.uniform(1, 97, (G, 2))
    print(kernel(bb_, gtb)[:20])
